# revision 9
# baseline (speedup 1.0000x reference)
"""LSTM autoencoder (encoder LSTM + autoregressive decoder LSTM + linear head)
as a Bass/Tile kernel for Trainium2, data-parallel over 8 NeuronCores.

Layout/algebra notes
--------------------
B=256 batch is sharded 32 per core. Per core, per LSTM step the gate
pre-activations are computed as one PE pass:

    gates[b, :] = [h_{t-1}; x_t; 1] @ [WhhT; WihT; bias]      (K = 512+8+1)

i.e. the input projection and both biases ride along as extra contraction
rows, so there is no separate input-projection matmul and no bias add.

The 2048 gate columns are permuted host-side into 4 hidden-slices of 512
columns, each ordered [f i g o] x 128. The 4 N-chunks of 512 are issued as
4 concurrent column-tiled matmuls (tile_position cols 0/32/64/96), so the
PSUM gates tile is [128 partitions = slice*32+batch, 512 free] and every
elementwise op runs on all 128 partitions.

The cell state c is kept in normal layout [slice*batch, hidden_in_slice].
m1 = sig(f)*c and m2 = sig(i)*tanh(g) are PE-transposed *accumulating* into
one PSUM tile, giving cT = (m1+m2)^T without a serial DVE add in the
critical path; tanh(cT) and hT = sig(o)^T * tanh(cT) then happen directly in
transposed space, so the hT chunks needed as next-step matmul stationaries
are plain AP slices (no extra transpose).

The decoder feedback path is linear, so it is folded:
    xg_{t} = relu(h_t) @ (lin_W^T dec_Wih^T) + (lin_b dec_Wih^T + dec_bih)
which makes every decoder step: bias-row group + 4 h groups + 4 relu(h)
groups of the same fused gate matmul, plus a tiny output projection.

All matmuls run as float32r (fp32 bits, relaxed-precision PE mode, 1
cycle/row at N=512 vs 4 for strict fp32).
"""

import sys

import numpy as np

import concourse.bass as bass  # noqa: F401  (bass types used via bacc/tile)
import concourse.mybir as mybir
import concourse.tile as tile
from concourse import bacc
from concourse.bass_utils import run_bass_kernel_spmd
from concourse.masks import make_identity

F32 = mybir.dt.float32
F32R = mybir.dt.float32r
AF = mybir.ActivationFunctionType
ALU = mybir.AluOpType

B, I, H = 256, 8, 512
NCORES = 8
BC = B // NCORES  # 32 batch per core
G = 4 * H  # 2048 gate columns
NSL = 4  # hidden slices of 128
XS_CHUNK = 8  # steps per xs DMA


def _gate_col_perm() -> np.ndarray:
    """Column permutation mapping our device gate layout to the original
    [i f g o] x 512 gate-row indexing of the PyTorch-style weights."""
    bases = {"i": 0, "f": H, "g": 2 * H, "o": 3 * H}
    perm = np.empty(G, dtype=np.int64)
    idx = 0
    for gname in ("f", "i", "g", "o"):
        for h in range(H):
            perm[idx] = bases[gname] + h
            idx += 1
    return perm


def _prep_host(inputs: dict[str, np.ndarray], S: int, T: int):
    """Build the shared (weight) arrays and the per-core xs arrays."""
    f32 = np.float32
    perm = _gate_col_perm()

    def gperm(mat_2048_cols):  # [.., 2048] -> permuted columns
        return np.ascontiguousarray(mat_2048_cols[..., perm]).astype(f32)

    enc_WhhT = gperm(inputs["enc_Whh"].T)  # [512, 2048]
    dec_WhhT = gperm(inputs["dec_Whh"].T)
    m2w = gperm(inputs["lin_W"].T @ inputs["dec_Wih"].T)  # [512, 2048]

    wx_enc = np.concatenate(
        [
            gperm(inputs["enc_Wih"].T),  # [8, 2048]
            gperm(inputs["enc_bih"] + inputs["enc_bhh"])[None, :],
        ],
        axis=0,
    )  # [9, 2048]
    wx_dec = np.concatenate(
        [
            gperm(inputs["dec_Wih"].T),
            gperm(inputs["dec_bih"] + inputs["dec_bhh"])[None, :],
        ],
        axis=0,
    )  # [9, 2048]
    brow = gperm(inputs["lin_b"] @ inputs["dec_Wih"].T + inputs["dec_bih"] + inputs["dec_bhh"])[
        None, :
    ]  # [1, 2048]

    # lin head: lhsT chunks [128, 8] packed side by side -> [128, 32]
    wout = (
        np.ascontiguousarray(inputs["lin_W"].T)
        .astype(f32)
        .reshape(NSL, 128, I)
        .transpose(1, 0, 2)
        .reshape(128, NSL * I)
    )
    bout = inputs["lin_b"].astype(f32)[None, :]  # [1, 8]

    shared = {
        "wenc": enc_WhhT.reshape(NSL, 128, G),
        "wxenc": wx_enc.astype(f32),
        "wdec": dec_WhhT.reshape(NSL, 128, G),
        "wm2": m2w.reshape(NSL, 128, G),
        "wxdec": wx_dec.astype(f32),
        "brow": brow.astype(f32),
        "wout": wout,
        "bout": bout,
    }
    shared["cones"] = np.ones((1, BC), f32)
    shared = {k: np.ascontiguousarray(v, dtype=f32) for k, v in shared.items()}

    seq = np.asarray(inputs["seq"], dtype=f32)  # [B, S, I]
    per_core = []
    for c in range(NCORES):
        seq_c = seq[c * BC : (c + 1) * BC]  # [32, S, 8]
        xs8 = seq_c.transpose(2, 1, 0).reshape(I, S * BC)  # [8, S*32]
        xs = np.concatenate([xs8, np.ones((1, S * BC), f32)], axis=0)  # [9, S*32]
        per_core.append(np.ascontiguousarray(xs, dtype=f32))
    return shared, per_core


def build_program(S: int, T: int):
    """Build the (single-core SPMD) Bass program. Returns the compiled nc."""
    nc = bacc.Bacc("TRN2", target_bir_lowering=False, debug=False)

    d_wenc = nc.dram_tensor("wenc", [NSL, 128, G], F32R, kind="ExternalInput")
    d_wxenc = nc.dram_tensor("wxenc", [9, G], F32R, kind="ExternalInput")
    d_wdec = nc.dram_tensor("wdec", [NSL, 128, G], F32R, kind="ExternalInput")
    d_wm2 = nc.dram_tensor("wm2", [NSL, 128, G], F32R, kind="ExternalInput")
    d_wxdec = nc.dram_tensor("wxdec", [9, G], F32R, kind="ExternalInput")
    d_brow = nc.dram_tensor("brow", [1, G], F32R, kind="ExternalInput")
    d_wout = nc.dram_tensor("wout", [128, NSL * I], F32R, kind="ExternalInput")
    d_bout = nc.dram_tensor("bout", [1, I], F32R, kind="ExternalInput")
    d_xs = nc.dram_tensor("xs", [9, S * BC], F32R, kind="ExternalInput")
    d_ones = nc.dram_tensor("cones", [1, BC], F32R, kind="ExternalInput")
    d_yout = nc.dram_tensor("yout", [I, T * BC], F32, kind="ExternalOutput")

    with tile.TileContext(nc) as tc:
        with (
            tc.tile_pool(name="const", bufs=1) as cpool,
            tc.tile_pool(name="xsp", bufs=3) as xspool,
            tc.tile_pool(name="work", bufs=2) as wpool,
            tc.tile_pool(name="ps", bufs=2, space="PSUM") as ppool,
        ):
            ident32 = cpool.tile([32, 32], F32, name="ident32")
            make_identity(nc, ident32)
            ones = cpool.tile([1, BC], F32R, name="ones")
            nc.sync.dma_start(ones, d_ones[:])

            wx_enc = cpool.tile([9, G], F32R, name="wx_enc")
            nc.sync.dma_start(wx_enc, d_wxenc[:])
            wenc = []
            for k in range(NSL):
                w = cpool.tile([128, G], F32R, name=f"wenc{k}")
                nc.sync.dma_start(w, d_wenc[k])
                wenc.append(w)
            wdec = []
            for k in range(NSL):
                w = cpool.tile([128, G], F32R, name=f"wdec{k}")
                nc.sync.dma_start(w, d_wdec[k])
                wdec.append(w)
            wm2 = []
            for k in range(NSL):
                w = cpool.tile([128, G], F32R, name=f"wm2{k}")
                nc.sync.dma_start(w, d_wm2[k])
                wm2.append(w)
            wx_dec = cpool.tile([9, G], F32R, name="wx_dec")
            nc.sync.dma_start(wx_dec, d_wxdec[:])
            brow = cpool.tile([1, G], F32R, name="brow")
            nc.sync.dma_start(brow, d_brow[:])
            wout = cpool.tile([128, NSL * I], F32R, name="wout")
            nc.sync.dma_start(wout, d_wout[:])
            bout = cpool.tile([1, I], F32R, name="bout")
            nc.sync.dma_start(bout, d_bout[:])
            outbuf = cpool.tile([I, T * BC], F32, name="outbuf")

            state: dict[str, bass.AP | None] = {"hT": None, "c": None, "reluHT": None}

            def mm_groups(gps, groups):
                ng = len(groups)
                for gi, (lhsT, rhs) in enumerate(groups):
                    for n in range(NSL):
                        nc.tensor.matmul(
                            gps[:, 512 * n : 512 * (n + 1)],
                            lhsT,
                            rhs[:, 512 * n : 512 * (n + 1)],
                            start=(gi == 0),
                            stop=(gi == ng - 1),
                        )

            def h_groups(w_chunks):
                hT = state["hT"]
                return [(hT[:, 32 * k : 32 * (k + 1)], w_chunks[k]) for k in range(NSL)]

            def relu_groups():
                rh = state["reluHT"]
                return [(rh[:, 32 * k : 32 * (k + 1)], wm2[k]) for k in range(NSL)]

            def cell_tail(gps, first: bool):
                # gps: [32, 2048] over 4 banks, columns [f i g o] x 512
                sf = wpool.tile([32, 512], F32, tag="sf")
                nc.scalar.activation(sf, gps[:, 0:512], AF.Sigmoid)
                si = wpool.tile([32, 512], F32, tag="si")
                nc.scalar.activation(si, gps[:, 512:1024], AF.Sigmoid)
                tg = wpool.tile([32, 512], F32, tag="tg")
                nc.scalar.activation(tg, gps[:, 1024:1536], AF.Tanh)
                so = wpool.tile([32, 512], F32, tag="so")
                nc.scalar.activation(so, gps[:, 1536:2048], AF.Sigmoid)

                m2 = wpool.tile([32, 512], F32, tag="m2")
                nc.vector.tensor_tensor(m2, si, tg, ALU.mult)
                # One PSUM zero-region covers the whole [128, 128] tile, so
                # exactly one start=True (first writer, lazily zeroes the
                # region) and one stop=True (last writer) per tile per step.
                cT = ppool.tile([128, 128], F32, tag="cT", bufs=1)
                if first:
                    for k in range(NSL):
                        nc.tensor.matmul(
                            cT[:, 32 * k : 32 * (k + 1)],
                            m2[:, 128 * k : 128 * (k + 1)],
                            ident32,
                            is_transpose=True,
                            start=(k == 0),
                            stop=(k == NSL - 1),
                        )
                    state["c"] = m2
                else:
                    m1 = wpool.tile([32, 512], F32, tag="m1")
                    nc.vector.tensor_tensor(m1, sf, state["c"], ALU.mult)
                    c_new = wpool.tile([32, 512], F32, tag="c")
                    nc.vector.tensor_tensor(c_new, m1, m2, ALU.add)
                    for k in range(NSL):
                        nc.tensor.matmul(
                            cT[:, 32 * k : 32 * (k + 1)],
                            m1[:, 128 * k : 128 * (k + 1)],
                            ident32,
                            is_transpose=True,
                            start=(k == 0),
                            stop=False,
                        )
                    for k in range(NSL):
                        nc.tensor.matmul(
                            cT[:, 32 * k : 32 * (k + 1)],
                            m2[:, 128 * k : 128 * (k + 1)],
                            ident32,
                            is_transpose=True,
                            start=False,
                            stop=(k == NSL - 1),
                        )
                    state["c"] = c_new
                soT = ppool.tile([128, 128], F32, tag="soT", bufs=1)
                for k in range(NSL):
                    nc.tensor.matmul(
                        soT[:, 32 * k : 32 * (k + 1)],
                        so[:, 128 * k : 128 * (k + 1)],
                        ident32,
                        is_transpose=True,
                        start=(k == 0),
                        stop=(k == NSL - 1),
                    )
                tcT = wpool.tile([128, 128], F32, tag="tcT")
                nc.scalar.activation(tcT, cT, AF.Tanh)
                hT = wpool.tile([128, 128], F32R, tag="hT")
                nc.vector.tensor_tensor(hT, soT, tcT, ALU.mult)
                state["hT"] = hT

            # ---------------- encoder ----------------
            xs_tile = None
            for t in range(S):
                if t % XS_CHUNK == 0:
                    nsteps = min(XS_CHUNK, S - t)
                    xs_tile = xspool.tile([9, XS_CHUNK * BC], F32R, tag="xs")
                    nc.sync.dma_start(
                        xs_tile[:, : nsteps * BC],
                        d_xs[:, t * BC : (t + nsteps) * BC],
                    )
                u = t % XS_CHUNK
                x_lhsT = xs_tile[:, u * BC : (u + 1) * BC]
                gps = ppool.tile([32, 2048], F32, tag="g", bufs=1)
                groups = [(x_lhsT, wx_enc)]
                if t > 0:
                    groups += h_groups(wenc)
                mm_groups(gps, groups)
                cell_tail(gps, first=(t == 0))
                if t == S - 1:
                    x_last = x_lhsT  # [9, 32] slice of the live xs tile

            # ---------------- decoder ----------------
            for t in range(T):
                gps = ppool.tile([32, 2048], F32, tag="g", bufs=1)
                if t == 0:
                    groups = [(x_last, wx_dec)]
                else:
                    groups = [(ones, brow)]
                groups += h_groups(wdec)
                if t > 0:
                    groups += relu_groups()
                mm_groups(gps, groups)
                cell_tail(gps, first=False)

                reluHT = wpool.tile([128, 128], F32R, tag="reluHT")
                nc.vector.tensor_scalar_max(reluHT, state["hT"], 0.0)
                state["reluHT"] = reluHT

                ops = ppool.tile([I, BC], F32, tag="op")
                for k in range(NSL):
                    nc.tensor.matmul(
                        ops,
                        wout[:, I * k : I * (k + 1)],
                        reluHT[:, 32 * k : 32 * (k + 1)],
                        start=(k == 0),
                        stop=False,
                    )
                nc.tensor.matmul(ops, bout, ones, start=False, stop=True)
                nc.vector.tensor_copy(outbuf[:, BC * t : BC * (t + 1)], ops)

            nc.sync.dma_start(d_yout[:], outbuf)

    nc.compile()
    return nc


def kernel(**inputs) -> np.ndarray:
    inputs = {k: (np.asarray(v) if not np.isscalar(v) else v) for k, v in inputs.items()}
    seq = np.asarray(inputs["seq"], dtype=np.float32)
    S = seq.shape[1]
    T = int(inputs["horizon"])
    assert seq.shape == (B, S, I)

    shared, per_core_xs = _prep_host(inputs, S, T)
    nc = build_program(S, T)

    in_maps = [dict(shared, xs=per_core_xs[c]) for c in range(NCORES)]
    res = run_bass_kernel_spmd(nc, in_maps, core_ids=list(range(NCORES)))
    # Stash for harnesses that want profiling info (exec_time_ns, trace path).
    sys.modules[__name__]._last_results = res

    out = np.empty((B, T, I), dtype=np.float32)
    for c in range(NCORES):
        yc = res.results[c]["yout"]  # [8, T*32]
        out[c * BC : (c + 1) * BC] = yc.reshape(I, T, BC).transpose(2, 1, 0)
    return out


# revision 10
# speedup vs baseline: 1.4732x; 1.4732x over previous
"""LSTM autoencoder (encoder LSTM + autoregressive decoder LSTM + linear head)
as a Bass/Tile kernel for Trainium2, data-parallel over 8 NeuronCores.

Layout / algebra notes
----------------------
B=256 batch is sharded 32 per core. Per core, per LSTM step the gate
pre-activations are one fused PE pass:

    gates[b, :] = [h_{t-1}; x_t; 1] @ [WhhT; WihT; bias]      (K = 512+8+1)

The fp32 contraction is evaluated as bf16 hi/lo split products (bf16
products are exact in the PE's fp32 accumulator, so with W = Whi + Wlo and
h = Hhi + Hlo the three cross terms drop only Wlo@Hlo ~ 2^-16 relative).
TERMS picks the accuracy/speed tradeoff:
  3: Whi@Hhi + Wlo@Hhi + Whi@Hlo   (error ~1e-5)
  2: Whi@Hhi + Wlo@Hhi = W@Hhi     (h truncated to bf16, W exact)
  1: Whi@Hhi                       (plain bf16)

bf16 re-enables 4-way PE column tiling (tile_position cols 0/32/64/96,
which the fp32r weight path forbids): the 2048 gate columns are permuted
host-side into 4 hidden-slices of 512, each ordered [f i g o] x 128, and
each K-group issues its 4 N-chunks as concurrent column tiles. The PSUM
gates tile is [128 partitions = slice*32+batch, 512 free], so every
elementwise op runs on all 128 partitions.

The cell state c stays in normal layout [(slice,batch), j]. m1 = sig(f)*c
and m2 = sig(i)*tanh(g) are PE-transposed accumulating into one PSUM tile,
giving cT = (m1+m2)^T with no serial DVE add on the critical path;
tanh(cT) and hT = sig(o)^T * tanh(cT) happen in transposed space, so the
hT contraction chunks are plain AP column slices.

The decoder feedback path is linear and is folded:
    xg_t = relu(h_t) @ (lin_W^T dec_Wih^T) + (lin_b dec_Wih^T + dec_bih)
making each decoder step: bias group + h groups + relu(h) groups of the
same fused gate matmul, plus a small fp32r output projection.
"""

import sys

import ml_dtypes
import numpy as np

import concourse.bass as bass  # noqa: F401
import concourse.mybir as mybir
import concourse.tile as tile
from concourse import bacc
from concourse.bass_utils import run_bass_kernel_spmd
from concourse.masks import make_identity

F32 = mybir.dt.float32
F32R = mybir.dt.float32r
BF16 = mybir.dt.bfloat16
AF = mybir.ActivationFunctionType
ALU = mybir.AluOpType

B, I, H = 256, 8, 512
NCORES = 8
BC = B // NCORES  # 32 batch per core
G = 4 * H  # 2048 gate columns
NSL = 4  # hidden slices of 128
XS_CHUNK = 8  # steps per xs DMA

# Number of bf16 split terms for the gate matmuls (see module docstring).
TERMS = 3

BF = ml_dtypes.bfloat16


def _split(a: np.ndarray):
    hi = np.asarray(a, np.float32).astype(BF)
    lo = (np.asarray(a, np.float32) - hi.astype(np.float32)).astype(BF)
    return hi, lo


def _gate_col_perm() -> np.ndarray:
    """Device gate-column layout: slice n holds columns [f i g o] x 128 for
    hidden indices [128n, 128(n+1)); maps into the original [i f g o] x 512
    gate-row indexing of the PyTorch-style weights."""
    bases = {"i": 0, "f": H, "g": 2 * H, "o": 3 * H}
    perm = np.empty(G, dtype=np.int64)
    idx = 0
    for n in range(NSL):
        for gname in ("f", "i", "g", "o"):
            for j in range(128):
                perm[idx] = bases[gname] + n * 128 + j
                idx += 1
    return perm


def _prep_host(inputs: dict[str, np.ndarray], S: int, T: int):
    """Build the shared (weight) arrays and the per-core xs arrays."""
    f32 = np.float32
    perm = _gate_col_perm()
    a = lambda k: np.asarray(inputs[k], f32)

    def gperm(mat):  # [.., 2048] -> permuted columns, fp32
        return np.ascontiguousarray(np.asarray(mat, f32)[..., perm])

    enc_WhhT = gperm(a("enc_Whh").T)  # [512, 2048]
    dec_WhhT = gperm(a("dec_Whh").T)
    m2w = gperm(a("lin_W").T @ a("dec_Wih").T)  # [512, 2048]

    wx_enc = np.concatenate(
        [gperm(a("enc_Wih").T), gperm(a("enc_bih") + a("enc_bhh"))[None]], axis=0
    )  # [9, 2048]
    wx_dec = np.concatenate(
        [gperm(a("dec_Wih").T), gperm(a("dec_bih") + a("dec_bhh"))[None]], axis=0
    )
    brow2 = np.stack(
        _split(gperm(a("lin_b") @ a("dec_Wih").T + a("dec_bih") + a("dec_bhh")))
    )  # [2, 2048] bf16: bias hi + bias lo rows

    # lin head (fp32r): lhsT chunks [128, 8] packed side by side -> [128, 32]
    wout = (
        np.ascontiguousarray(a("lin_W").T)
        .reshape(NSL, 128, I)
        .transpose(1, 0, 2)
        .reshape(128, NSL * I)
    )

    shared: dict[str, np.ndarray] = {
        "wout": np.ascontiguousarray(wout, f32),
        "bout": np.ascontiguousarray(a("lin_b").reshape(I, 1)),
        "brow2": np.ascontiguousarray(brow2),
        "ones2": np.ones((2, BC), BF),
    }
    for name, w in [("wenc", enc_WhhT), ("wdec", dec_WhhT), ("wm2", m2w)]:
        hi, lo = _split(w)
        shared[name + "h"] = np.ascontiguousarray(hi.reshape(NSL, 128, G))
        shared[name + "l"] = np.ascontiguousarray(lo.reshape(NSL, 128, G))
    for name, w in [("wxenc", wx_enc), ("wxdec", wx_dec)]:
        hi, lo = _split(w)
        shared[name + "h"] = np.ascontiguousarray(hi)
        shared[name + "l"] = np.ascontiguousarray(lo)

    seq = np.asarray(inputs["seq"], dtype=f32)  # [B, S, I]
    per_core = []
    for c in range(NCORES):
        seq_c = seq[c * BC : (c + 1) * BC]  # [32, S, 8]
        xs8 = seq_c.transpose(2, 1, 0).reshape(I, S * BC)  # [8, S*32]
        xs = np.concatenate([xs8, np.ones((1, S * BC), f32)], axis=0)  # [9, S*32]
        hi, lo = _split(xs)
        per_core.append(
            {"xsh": np.ascontiguousarray(hi), "xsl": np.ascontiguousarray(lo)}
        )
    return shared, per_core


def build_program(S: int, T: int, terms: int | None = None):
    if terms is None:
        terms = TERMS
    nc = bacc.Bacc("TRN2", target_bir_lowering=False, debug=False)

    din = {}
    for name in ("wenc", "wdec", "wm2"):
        for sfx in ("h", "l"):
            din[name + sfx] = nc.dram_tensor(
                name + sfx, [NSL, 128, G], BF16, kind="ExternalInput"
            )
    for name in ("wxench", "wxencl", "wxdech", "wxdecl"):
        din[name] = nc.dram_tensor(name, [9, G], BF16, kind="ExternalInput")
    din["brow2"] = nc.dram_tensor("brow2", [2, G], BF16, kind="ExternalInput")
    din["ones2"] = nc.dram_tensor("ones2", [2, BC], BF16, kind="ExternalInput")
    din["wout"] = nc.dram_tensor("wout", [128, NSL * I], F32R, kind="ExternalInput")
    din["bout"] = nc.dram_tensor("bout", [I, 1], F32, kind="ExternalInput")
    din["xsh"] = nc.dram_tensor("xsh", [9, S * BC], BF16, kind="ExternalInput")
    din["xsl"] = nc.dram_tensor("xsl", [9, S * BC], BF16, kind="ExternalInput")
    d_yout = nc.dram_tensor("yout", [I, T * BC], F32, kind="ExternalOutput")

    with tile.TileContext(nc) as tc:
        with (
            tc.tile_pool(name="const", bufs=1) as cpool,
            tc.tile_pool(name="xsp", bufs=3) as xspool,
            tc.tile_pool(name="work", bufs=2) as wpool,
            tc.tile_pool(name="ps", bufs=2, space="PSUM") as ppool,
        ):
            ident = cpool.tile([128, 128], F32, name="ident")
            make_identity(nc, ident)

            def load(name, shape, dt):
                t = cpool.tile(shape, dt, name=name)
                nc.sync.dma_start(t, din[name][:])
                return t

            def load_chunks(name):
                out = []
                for sfx in ("h", "l"):
                    row = []
                    for k in range(NSL):
                        t = cpool.tile([128, G], BF16, name=f"{name}{sfx}{k}")
                        nc.sync.dma_start(t, din[name + sfx][k])
                        row.append(t)
                    out.append(row)
                return out

            wx_e = [load("wxench", [9, G], BF16)]
            wx_d = [load("wxdech", [9, G], BF16)]
            if terms >= 2:
                wx_e.append(load("wxencl", [9, G], BF16))
                wx_d.append(load("wxdecl", [9, G], BF16))
            wenc_hl = load_chunks("wenc")
            wdec_hl = load_chunks("wdec")
            wm2_hl = load_chunks("wm2")
            brow2 = load("brow2", [2, G], BF16)
            ones2 = load("ones2", [2, BC], BF16)
            wout = load("wout", [128, NSL * I], F32R)
            bout = load("bout", [I, 1], F32)
            outbuf = cpool.tile([I, T * BC], F32, name="outbuf")

            state: dict = {"Hh": None, "Hl": None, "c": None, "Rh": None, "Rl": None}

            def mm_groups(gps, groups):
                """groups: list of (lhsT, rhs); each issued as 4 concurrent
                column tiles over the 4 N-chunks of 512."""
                ng = len(groups)
                for gi, (lhsT, rhs) in enumerate(groups):
                    for n in range(NSL):
                        nc.tensor.matmul(
                            gps[32 * n : 32 * (n + 1), :],
                            lhsT,
                            rhs[:, 512 * n : 512 * (n + 1)],
                            start=(gi == 0),
                            stop=(gi == ng - 1),
                            tile_position=(0, 32 * n),
                            # The 4 column tiles accumulate into disjoint
                            # 32-partition slices of one bank; the sim's
                            # group-started tracker is per-bank and would
                            # false-positive on the per-tile starts.
                            skip_group_check=True,
                        )

            def split_products(w_hl, Hh, Hl):
                prods = [(w_hl[0], Hh)]
                if terms >= 2:
                    prods.append((w_hl[1], Hh))
                if terms >= 3:
                    prods.append((w_hl[0], Hl))
                groups = []
                for w_chunks, hT in prods:
                    for k in range(NSL):
                        groups.append((hT[:, 32 * k : 32 * (k + 1)], w_chunks[k]))
                return groups

            def x_products(xh, xl, wx):
                groups = [(xh, wx[0])]
                if terms >= 2:
                    groups.append((xh, wx[1]))
                if terms >= 3:
                    groups.append((xl, wx[0]))
                return groups

            def cell_tail(gps, first: bool, want_f32h: bool):
                sfi = wpool.tile([128, 256], F32, tag="sfi")
                nc.scalar.activation(sfi, gps[:, 0:256], AF.Sigmoid)
                tg = wpool.tile([128, 128], F32, tag="tg")
                nc.scalar.activation(tg, gps[:, 256:384], AF.Tanh)
                so = wpool.tile([128, 128], F32, tag="so")
                nc.scalar.activation(so, gps[:, 384:512], AF.Sigmoid)

                m2 = wpool.tile([128, 128], F32, tag="m2")
                nc.vector.tensor_tensor(m2, sfi[:, 128:256], tg, ALU.mult)
                cT = ppool.tile([128, 128], F32, tag="cT", bufs=1)
                if first:
                    nc.tensor.matmul(cT, m2, ident, is_transpose=True, start=True, stop=True)
                    state["c"] = m2
                else:
                    m1 = wpool.tile([128, 128], F32, tag="m1")
                    nc.vector.tensor_tensor(m1, sfi[:, 0:128], state["c"], ALU.mult)
                    c_new = wpool.tile([128, 128], F32, tag="c")
                    nc.vector.tensor_tensor(c_new, m1, m2, ALU.add)
                    nc.tensor.matmul(cT, m1, ident, is_transpose=True, start=True, stop=False)
                    nc.tensor.matmul(cT, m2, ident, is_transpose=True, start=False, stop=True)
                    state["c"] = c_new
                soT = ppool.tile([128, 128], F32, tag="soT", bufs=1)
                nc.tensor.matmul(soT, so, ident, is_transpose=True, start=True, stop=True)
                tcT = wpool.tile([128, 128], F32, tag="tcT")
                nc.scalar.activation(tcT, cT, AF.Tanh)

                hT = None
                if want_f32h or terms >= 3:
                    hT = wpool.tile([128, 128], F32, tag="hT")
                    nc.vector.tensor_tensor(hT, soT, tcT, ALU.mult)
                    Hh = wpool.tile([128, 128], BF16, tag="Hh")
                    nc.vector.tensor_copy(Hh, hT)
                else:
                    Hh = wpool.tile([128, 128], BF16, tag="Hh")
                    nc.vector.tensor_tensor(Hh, soT, tcT, ALU.mult)
                if terms >= 3:
                    Hl = wpool.tile([128, 128], BF16, tag="Hl")
                    nc.vector.tensor_tensor(Hl, hT, Hh, ALU.subtract)
                else:
                    Hl = None
                state["Hh"], state["Hl"] = Hh, Hl
                return hT

            # ---------------- encoder ----------------
            xsh_t = xsl_t = None
            for t in range(S):
                if t % XS_CHUNK == 0:
                    nsteps = min(XS_CHUNK, S - t)
                    xsh_t = xspool.tile([9, XS_CHUNK * BC], BF16, tag="xsh")
                    nc.sync.dma_start(
                        xsh_t[:, : nsteps * BC], din["xsh"][:, t * BC : (t + nsteps) * BC]
                    )
                    if terms >= 3:
                        xsl_t = xspool.tile([9, XS_CHUNK * BC], BF16, tag="xsl")
                        nc.sync.dma_start(
                            xsl_t[:, : nsteps * BC],
                            din["xsl"][:, t * BC : (t + nsteps) * BC],
                        )
                u = t % XS_CHUNK
                xh = xsh_t[:, u * BC : (u + 1) * BC]
                xl = xsl_t[:, u * BC : (u + 1) * BC] if terms >= 3 else None
                gps = ppool.tile([128, 512], F32, tag="g")
                groups = x_products(xh, xl, wx_e)
                if t > 0:
                    groups += split_products(wenc_hl, state["Hh"], state["Hl"])
                mm_groups(gps, groups)
                cell_tail(gps, first=(t == 0), want_f32h=False)
                if t == S - 1:
                    x_last_h, x_last_l = xh, xl

            # ---------------- decoder ----------------
            for t in range(T):
                gps = ppool.tile([128, 512], F32, tag="g")
                if t == 0:
                    groups = x_products(x_last_h, x_last_l, wx_d)
                else:
                    groups = [(ones2, brow2)]
                groups += split_products(wdec_hl, state["Hh"], state["Hl"])
                if t > 0:
                    groups += split_products(wm2_hl, state["Rh"], state["Rl"])
                mm_groups(gps, groups)
                hT = cell_tail(gps, first=False, want_f32h=True)

                # relu path: fp32r copy feeds the output projection; bf16
                # split feeds next step's wm2 groups.
                rel = wpool.tile([128, 128], F32R, tag="rel")
                nc.vector.tensor_scalar_max(rel, hT, 0.0)
                Rh = wpool.tile([128, 128], BF16, tag="Rh")
                nc.vector.tensor_copy(Rh, rel)
                if terms >= 3:
                    Rl = wpool.tile([128, 128], BF16, tag="Rl")
                    nc.vector.tensor_tensor(Rl, rel, Rh, ALU.subtract)
                else:
                    Rl = None
                state["Rh"], state["Rl"] = Rh, Rl

                ops = ppool.tile([I, BC], F32, tag="op")
                for k in range(NSL):
                    nc.tensor.matmul(
                        ops,
                        wout[:, I * k : I * (k + 1)],
                        rel[:, 32 * k : 32 * (k + 1)],
                        start=(k == 0),
                        stop=(k == NSL - 1),
                    )
                # out = outT + lin_b: per-partition scalar add during copyback
                nc.vector.tensor_scalar(
                    outbuf[:, BC * t : BC * (t + 1)], ops, bout, None, ALU.add
                )

            nc.sync.dma_start(d_yout[:], outbuf)

    nc.compile()
    return nc


def kernel(**inputs) -> np.ndarray:
    seq = np.asarray(inputs["seq"], dtype=np.float32)
    S = seq.shape[1]
    T = int(inputs["horizon"])
    assert seq.shape == (B, S, I)

    shared, per_core = _prep_host(inputs, S, T)
    nc = build_program(S, T)

    in_maps = [dict(shared, **per_core[c]) for c in range(NCORES)]
    res = run_bass_kernel_spmd(nc, in_maps, core_ids=list(range(NCORES)))
    sys.modules[__name__]._last_results = res

    out = np.empty((B, T, I), dtype=np.float32)
    for c in range(NCORES):
        yc = res.results[c]["yout"]  # [8, T*32]
        out[c * BC : (c + 1) * BC] = yc.reshape(I, T, BC).transpose(2, 1, 0)
    return out
